# revision 8
# baseline (speedup 1.0000x reference)
"""Trainium2 Bass kernel for nn_MemoryMultiAttention.

out = x + softmax((x @ Wq + bq) K^T / sqrt(D)) V   per head, tiny shared
memory bank (M=64 slots), H=4 heads of dh=16, D=64.

Strategy:
  * Host folds the Q projection into the score matrix:
        scores[t, h, m] = x[t, :] @ A_h[:, m] + c_h[m]
    with A_h = Wq_h @ K_h^T / 8 (64x64), c_h = bq_h @ K_h^T / 8.
  * Data-parallel over 8 cores: each core handles 1/8 of the B*L*N tokens.
  * The host supplies, per core, both the fp32 tokens (for the residual)
    and a bf16 *transposed* copy laid out [128 = 2 token-halves x 64 d,
    cols] so the scores matmul can contract over d directly; two 64-row
    groups of the PE run concurrently.
  * On device (per supertile of 1024 tokens):
      - TensorE: scoresT[hm, t] = A_pair^T @ xT   (psum [128, 2, 512])
      - ACT: exp(scores + c) with per-partition bias fused; bf16 out
      - TensorE: read_u[t, 0:64] + per-head sumexp[t, 64:68] in one
        accumulated matmul against an augmented block-diagonal V
      - DVE: reciprocal of sums, normalize, add fp32 residual x
  * Token order inside a supertile is permuted so every DMA is 2KB-
    contiguous per partition; the host applies the inverse permutation.
"""

import math
from contextlib import ExitStack

import ml_dtypes
import numpy as np

import concourse.bass as bass
import concourse.mybir as mybir
import concourse.tile as tile
from concourse import bacc
from concourse.bass_utils import run_bass_kernel_spmd

B, L, N, D = 16, 24, 325, 64
M, H = 64, 4
DH = D // H
TOK = B * L * N  # 124800
NCORES = 8
NT = 16384  # padded tokens per core (124800/8 = 15600 -> 16*1024)
NSUP = 16
TS = 1024  # supertile tokens
CH = TS // 128  # 8 chunks of 128 tokens

F32 = mybir.dt.float32
BF16 = mybir.dt.bfloat16

# set by test.py to collect a profile
TRACE = False
LAST_RESULTS = None

_cached_nc = None


def _build_program():
    global _cached_nc
    if _cached_nc is not None:
        return _cached_nc

    nc = bacc.Bacc(
        "TRN2", target_bir_lowering=False, debug=False, num_devices=NCORES
    )
    x_in = nc.declare_dram_parameter("x", [NT, D], F32, isOutput=False)
    xt_in = nc.declare_dram_parameter("xt", [128, NT // 2], BF16, isOutput=False)
    a_in = nc.declare_dram_parameter("a", [128, 2, 128], BF16, isOutput=False)
    c_in = nc.declare_dram_parameter("c", [128, 2], F32, isOutput=False)
    v_in = nc.declare_dram_parameter("v", [128, 2, 68], BF16, isOutput=False)
    y_out = nc.declare_dram_parameter("y", [NT, D], F32, isOutput=True)

    with ExitStack() as ctx:
        tc = ctx.enter_context(tile.TileContext(nc))
        const_pool = ctx.enter_context(tc.tile_pool(name="const", bufs=1))
        xin_pool = ctx.enter_context(tc.tile_pool(name="xin", bufs=4))
        xt_pool = ctx.enter_context(tc.tile_pool(name="xt", bufs=4))
        exp_pool = ctx.enter_context(tc.tile_pool(name="expt", bufs=4))
        o32_pool = ctx.enter_context(tc.tile_pool(name="o32", bufs=3))
        out_pool = ctx.enter_context(tc.tile_pool(name="outp", bufs=3))
        rec_pool = ctx.enter_context(tc.tile_pool(name="recip", bufs=3))
        psS_pool = ctx.enter_context(tc.tile_pool(name="psS", bufs=3, space="PSUM"))
        psR_pool = ctx.enter_context(tc.tile_pool(name="psR", bufs=1, space="PSUM"))

        # constants, loaded once
        a_t = const_pool.tile([128, 2, 128], BF16)
        nc.sync.dma_start(a_t[:, :, :], a_in[:, :, :])
        c_t = const_pool.tile([128, 2], F32)
        nc.sync.dma_start(c_t[:, :], c_in[:, :])
        v_t = const_pool.tile([128, 2, 68], BF16)
        nc.sync.dma_start(v_t[:, :, :], v_in[:, :, :])

        x32_pair = xt_pair = outt_pair = None
        for s in range(NSUP):
            # device token f (col of xt) = 512c + 128k + p; x/y rows are
            # host-permuted so row 1024s + 8p + 4c + k = device token f
            half = s % 2
            if half == 0:
                # one DMA covers two supertiles: bigger descriptors,
                # half the sequencer issue cost
                x32_pair = xin_pool.tile([128, 2, CH * D], F32, tag="x32")
                nc.sync.dma_start(
                    x32_pair[:, :, :],
                    x_in[TS * s : TS * (s + 2), :].rearrange(
                        "(u p q) d -> p u (q d)", u=2, p=128
                    ),
                )
                xt_pair = xt_pool.tile([128, 2, 512], BF16, tag="xt")
                nc.sync.dma_start(
                    xt_pair[:, :, :],
                    xt_in[:, 512 * s : 512 * (s + 2)].rearrange(
                        "p (u f) -> p u f", u=2
                    ),
                )
            x32 = x32_pair[:, half]
            xt = xt_pair[:, half]

            # scoresT: psS[pp][hm, (c, f)]
            psS = []
            expt = []
            for pp in range(2):
                ps = psS_pool.tile([128, 2, 512], F32, tag="psS")
                for c in range(2):
                    nc.tensor.matmul(
                        ps[:, c, :],
                        a_t[64 * c : 64 * (c + 1), pp, :],
                        xt[64 * c : 64 * (c + 1), :],
                        start=True,
                        stop=True,
                    )
                et = exp_pool.tile([128, 2, 512], BF16, tag="expt")
                nc.scalar.activation(
                    et[:, :, :],
                    ps[:, :, :],
                    mybir.ActivationFunctionType.Exp,
                    bias=c_t[:, pp : pp + 1],
                )
                psS.append(ps)
                expt.append(et)

            # read: chunk cc = 4c + k lives at psR[:, c, k, :];
            # cols 0:64 = read_u, 64:68 = per-head sumexp
            psR = psR_pool.tile([128, 2, 4, 128], F32, tag="psR")
            for cc in range(CH):
                c, k = cc // 4, cc % 4
                for pp in range(2):
                    nc.tensor.matmul(
                        psR[:, c, k, 0:68],
                        expt[pp][:, c, 128 * k : 128 * (k + 1)],
                        v_t[:, pp, :],
                        start=(pp == 0),
                        stop=(pp == 1),
                    )

            rec = rec_pool.tile([128, 2, 4, 4], F32, tag="rec")
            nc.vector.reciprocal(rec[:, :, :, :], psR[:, :, :, 64:68])

            o32 = o32_pool.tile([128, 2, 4, 4, 16], F32, tag="o32")
            nc.vector.tensor_mul(
                o32[:, :, :, :, :],
                psR[:, :, :, 0:64].rearrange("p b k (h e) -> p b k h e", e=16),
                rec[:, :, :, :].unsqueeze(4).broadcast_to((128, 2, 4, 4, 16)),
            )

            if half == 0:
                outt_pair = out_pool.tile([128, 2, CH * D], F32, tag="outt")
            nc.vector.tensor_add(
                outt_pair[:, half],
                o32[:, :, :, :, :].rearrange("p b k h e -> p (b k h e)"),
                x32[:, :],
            )
            if half == 1:
                nc.sync.dma_start(
                    y_out[TS * (s - 1) : TS * (s + 1), :].rearrange(
                        "(u p q) d -> p u (q d)", u=2, p=128
                    ),
                    outt_pair[:, :, :],
                )

    nc.compile()
    _cached_nc = nc
    return nc


def _host_constants(memory_bank, Wq, bq, Wk, bk, Wv, bv):
    mb = np.asarray(memory_bank, np.float32)
    Wq = np.asarray(Wq, np.float32)
    bq = np.asarray(bq, np.float32)
    Wk = np.asarray(Wk, np.float32)
    bk = np.asarray(bk, np.float32)
    Wv = np.asarray(Wv, np.float32)
    bv = np.asarray(bv, np.float32)

    K = mb @ Wk + bk  # [M, D]
    V = mb @ Wv + bv  # [M, D]
    scale = 1.0 / math.sqrt(D)

    # a_np[64c + d, pp, j]: A for head (2pp + j//64), slot j%64, replicated c
    a_np = np.zeros((128, 2, 128), np.float32)
    c_np = np.zeros((128, 2), np.float32)
    v_np = np.zeros((128, 2, 68), np.float32)
    for h in range(H):
        Kh = K[:, h * DH : (h + 1) * DH]  # [M, dh]
        Vh = V[:, h * DH : (h + 1) * DH]  # [M, dh]
        Ah = (Wq[:, h * DH : (h + 1) * DH] @ Kh.T) * scale  # [D, M]
        ch = (bq[h * DH : (h + 1) * DH] @ Kh.T) * scale  # [M]
        pp, half = h // 2, h % 2
        for c in range(2):
            a_np[64 * c : 64 * (c + 1), pp, 64 * half : 64 * (half + 1)] = Ah
        q0 = 64 * half
        c_np[q0 : q0 + 64, pp] = ch
        v_np[q0 : q0 + 64, pp, h * DH : (h + 1) * DH] = Vh
        v_np[q0 : q0 + 64, pp, 64 + h] = 1.0

    return (
        a_np.astype(ml_dtypes.bfloat16),
        c_np,
        v_np.astype(ml_dtypes.bfloat16),
    )


def kernel(x, memory_bank, Wq, bq, Wk, bk, Wv, bv):
    global LAST_RESULTS
    a_np, c_np, v_np = _host_constants(memory_bank, Wq, bq, Wk, bk, Wv, bv)

    x_np = np.ascontiguousarray(np.asarray(x, np.float32).reshape(TOK, D))
    x_pad = np.zeros((NCORES * NT, D), np.float32)
    x_pad[:TOK] = x_np
    x_pad = x_pad.reshape(NCORES, NSUP, 2, 4, 128, D)  # [n, s, c, k, p, d]

    # device-permuted fp32 tokens: row 1024s + 8p + 4c + k
    x_perm = np.ascontiguousarray(x_pad.transpose(0, 1, 4, 2, 3, 5)).reshape(
        NCORES, NT, D
    )
    # transposed bf16 tokens: xt[n, 64c + d, 512s + 128k + p]
    xt16 = np.ascontiguousarray(
        x_pad.astype(ml_dtypes.bfloat16).transpose(0, 2, 5, 1, 3, 4)
    ).reshape(NCORES, 128, NT // 2)

    in_maps = [
        {
            "x": x_perm[n],
            "xt": xt16[n],
            "a": a_np,
            "c": c_np,
            "v": v_np,
        }
        for n in range(NCORES)
    ]

    nc = _build_program()
    res = run_bass_kernel_spmd(nc, in_maps, list(range(NCORES)), trace=TRACE)
    LAST_RESULTS = res

    y = np.stack([res.results[n]["y"] for n in range(NCORES)], axis=0)
    # invert the per-supertile permutation: perm row = 8p + 4c + k
    y = y.reshape(NCORES, NSUP, 128, 2, 4, D).transpose(0, 1, 3, 4, 2, 5)
    y = np.ascontiguousarray(y).reshape(NCORES * NT, D)
    return y[:TOK].reshape(B, L, N, D)


# revision 11
# speedup vs baseline: 1.0659x; 1.0659x over previous
"""Trainium2 Bass kernel for nn_MemoryMultiAttention.

out = x + softmax((x @ Wq + bq) K^T / sqrt(D)) V   per head, tiny shared
memory bank (M=64 slots), H=4 heads of dh=16, D=64.

Strategy:
  * Host folds the Q projection into the score matrix:
        scores[t, h, m] = x[t, :] @ A_h[:, m] + c_h[m]
    with A_h = Wq_h @ K_h^T / 8 (64x64), c_h = bq_h @ K_h^T / 8.
  * Data-parallel over 8 cores: each core handles 1/8 of the B*L*N tokens.
  * The host supplies, per core, both the fp32 tokens (for the residual)
    and a bf16 *transposed* copy laid out [128 = 2 token-halves x 64 d,
    cols] so the scores matmul can contract over d directly; two 64-row
    groups of the PE run concurrently.
  * On device (per supertile of 1024 tokens):
      - TensorE: scoresT[hm, t] = A_pair^T @ xT   (psum [128, 2, 512])
      - ACT: exp(scores + c) with per-partition bias fused; bf16 out
      - TensorE: read_u[t, 0:64] + per-head sumexp[t, 64:68] in one
        accumulated matmul against an augmented block-diagonal V
      - DVE: reciprocal of sums, normalize, add fp32 residual x
  * Token order inside a supertile is permuted so every DMA is 2KB-
    contiguous per partition; the host applies the inverse permutation.
"""

import math
from contextlib import ExitStack

import ml_dtypes
import numpy as np

import concourse.bass as bass
import concourse.mybir as mybir
import concourse.tile as tile
from concourse import bacc
from concourse.bass_utils import run_bass_kernel_spmd

B, L, N, D = 16, 24, 325, 64
M, H = 64, 4
DH = D // H
TOK = B * L * N  # 124800
NCORES = 8
NT = 16384  # padded tokens per core (124800/8 = 15600 -> 16*1024)
NSUP = 16
TS = 1024  # supertile tokens
CH = TS // 128  # 8 chunks of 128 tokens

F32 = mybir.dt.float32
BF16 = mybir.dt.bfloat16

# set by test.py to collect a profile
TRACE = False
LAST_RESULTS = None

_cached_nc = None


def _build_program():
    global _cached_nc
    if _cached_nc is not None:
        return _cached_nc

    nc = bacc.Bacc(
        "TRN2", target_bir_lowering=False, debug=False, num_devices=NCORES
    )
    x_in = nc.declare_dram_parameter("x", [NT, D], F32, isOutput=False)
    xt_in = nc.declare_dram_parameter("xt", [128, NT // 2], BF16, isOutput=False)
    a_in = nc.declare_dram_parameter("a", [128, 2, 128], BF16, isOutput=False)
    c_in = nc.declare_dram_parameter("c", [128, 2], F32, isOutput=False)
    v_in = nc.declare_dram_parameter("v", [128, 2, 68], BF16, isOutput=False)
    y_out = nc.declare_dram_parameter("y", [NT, D], F32, isOutput=True)

    with ExitStack() as ctx:
        tc = ctx.enter_context(tile.TileContext(nc))
        const_pool = ctx.enter_context(tc.tile_pool(name="const", bufs=1))
        xin_pool = ctx.enter_context(tc.tile_pool(name="xin", bufs=4))
        xt_pool = ctx.enter_context(tc.tile_pool(name="xt", bufs=4))
        exp_pool = ctx.enter_context(tc.tile_pool(name="expt", bufs=6))
        o32_pool = ctx.enter_context(tc.tile_pool(name="o32", bufs=3))
        out_pool = ctx.enter_context(tc.tile_pool(name="outp", bufs=3))
        rec_pool = ctx.enter_context(tc.tile_pool(name="recip", bufs=3))
        psS_pool = ctx.enter_context(tc.tile_pool(name="psS", bufs=3, space="PSUM"))
        psR_pool = ctx.enter_context(tc.tile_pool(name="psR", bufs=1, space="PSUM"))

        # constants, loaded once
        a_t = const_pool.tile([128, 2, 128], BF16)
        nc.sync.dma_start(a_t[:, :, :], a_in[:, :, :])
        c_t = const_pool.tile([128, 2], F32)
        nc.sync.dma_start(c_t[:, :], c_in[:, :])
        v_t = const_pool.tile([128, 2, 68], BF16)
        nc.sync.dma_start(v_t[:, :, :], v_in[:, :, :])

        # software pipeline: scores/exp of supertile s are emitted before the
        # read/normalize phase of supertile s-1 so the PE starts the next
        # scores matmuls as soon as the previous exp drains, keeping ACT fed.
        stage = {}  # s -> (expt pair list, x32 AP)
        outp = {}  # pair idx -> outt tile

        def read_phase(s):
            expt, x32 = stage.pop(s)
            half = s % 2

            # read: chunk cc = 4c + k lives at psR[:, c, k, :];
            # cols 0:64 = read_u, 64:68 = per-head sumexp
            psR = psR_pool.tile([128, 2, 4, 128], F32, tag="psR")
            for cc in range(CH):
                c, k = cc // 4, cc % 4
                for pp in range(2):
                    nc.tensor.matmul(
                        psR[:, c, k, 0:68],
                        expt[pp][:, c, 128 * k : 128 * (k + 1)],
                        v_t[:, pp, :],
                        start=(pp == 0),
                        stop=(pp == 1),
                    )

            rec = rec_pool.tile([128, 2, 4, 4], F32, tag="rec")
            nc.vector.reciprocal(rec[:, :, :, :], psR[:, :, :, 64:68])

            o32 = o32_pool.tile([128, 2, 4, 4, 16], F32, tag="o32")
            nc.vector.tensor_mul(
                o32[:, :, :, :, :],
                psR[:, :, :, 0:64].rearrange("p b k (h e) -> p b k h e", e=16),
                rec[:, :, :, :].unsqueeze(4).broadcast_to((128, 2, 4, 4, 16)),
            )

            if half == 0:
                outp[s // 2] = out_pool.tile(
                    [128, 2, CH * D], F32, tag="outt", name=f"outt{s}"
                )
            nc.vector.tensor_add(
                outp[s // 2][:, half],
                o32[:, :, :, :, :].rearrange("p b k h e -> p (b k h e)"),
                x32[:, :],
            )
            if half == 1:
                nc.sync.dma_start(
                    y_out[TS * (s - 1) : TS * (s + 1), :].rearrange(
                        "(u p q) d -> p u (q d)", u=2, p=128
                    ),
                    outp.pop(s // 2)[:, :, :],
                )

        x32_pair = xt_pair = None
        for s in range(NSUP):
            # device token f (col of xt) = 512c + 128k + p; x/y rows are
            # host-permuted so row 1024s + 8p + 4c + k = device token f
            half = s % 2
            if half == 0:
                # one DMA covers two supertiles: bigger descriptors,
                # half the sequencer issue cost; xt first (needed first)
                xt_pair = xt_pool.tile([128, 2, 512], BF16, tag="xt")
                nc.sync.dma_start(
                    xt_pair[:, :, :],
                    xt_in[:, 512 * s : 512 * (s + 2)].rearrange(
                        "p (u f) -> p u f", u=2
                    ),
                )
                x32_pair = xin_pool.tile([128, 2, CH * D], F32, tag="x32")
                nc.sync.dma_start(
                    x32_pair[:, :, :],
                    x_in[TS * s : TS * (s + 2), :].rearrange(
                        "(u p q) d -> p u (q d)", u=2, p=128
                    ),
                )
            x32 = x32_pair[:, half]
            xt = xt_pair[:, half]

            # scoresT: psS[pp][hm, (c, f)]
            expt = []
            for pp in range(2):
                ps = psS_pool.tile([128, 2, 512], F32, tag="psS")
                for c in range(2):
                    nc.tensor.matmul(
                        ps[:, c, :],
                        a_t[64 * c : 64 * (c + 1), pp, :],
                        xt[64 * c : 64 * (c + 1), :],
                        start=True,
                        stop=True,
                    )
                et = exp_pool.tile([128, 2, 512], BF16, tag="expt")
                nc.scalar.activation(
                    et[:, :, :],
                    ps[:, :, :],
                    mybir.ActivationFunctionType.Exp,
                    bias=c_t[:, pp : pp + 1],
                )
                expt.append(et)
            stage[s] = (expt, x32)

            if s > 0:
                read_phase(s - 1)
        read_phase(NSUP - 1)

    nc.compile()
    _cached_nc = nc
    return nc


def _host_constants(memory_bank, Wq, bq, Wk, bk, Wv, bv):
    mb = np.asarray(memory_bank, np.float32)
    Wq = np.asarray(Wq, np.float32)
    bq = np.asarray(bq, np.float32)
    Wk = np.asarray(Wk, np.float32)
    bk = np.asarray(bk, np.float32)
    Wv = np.asarray(Wv, np.float32)
    bv = np.asarray(bv, np.float32)

    K = mb @ Wk + bk  # [M, D]
    V = mb @ Wv + bv  # [M, D]
    scale = 1.0 / math.sqrt(D)

    # a_np[64c + d, pp, j]: A for head (2pp + j//64), slot j%64, replicated c
    a_np = np.zeros((128, 2, 128), np.float32)
    c_np = np.zeros((128, 2), np.float32)
    v_np = np.zeros((128, 2, 68), np.float32)
    for h in range(H):
        Kh = K[:, h * DH : (h + 1) * DH]  # [M, dh]
        Vh = V[:, h * DH : (h + 1) * DH]  # [M, dh]
        Ah = (Wq[:, h * DH : (h + 1) * DH] @ Kh.T) * scale  # [D, M]
        ch = (bq[h * DH : (h + 1) * DH] @ Kh.T) * scale  # [M]
        pp, half = h // 2, h % 2
        for c in range(2):
            a_np[64 * c : 64 * (c + 1), pp, 64 * half : 64 * (half + 1)] = Ah
        q0 = 64 * half
        c_np[q0 : q0 + 64, pp] = ch
        v_np[q0 : q0 + 64, pp, h * DH : (h + 1) * DH] = Vh
        v_np[q0 : q0 + 64, pp, 64 + h] = 1.0

    return (
        a_np.astype(ml_dtypes.bfloat16),
        c_np,
        v_np.astype(ml_dtypes.bfloat16),
    )


def kernel(x, memory_bank, Wq, bq, Wk, bk, Wv, bv):
    global LAST_RESULTS
    a_np, c_np, v_np = _host_constants(memory_bank, Wq, bq, Wk, bk, Wv, bv)

    x_np = np.ascontiguousarray(np.asarray(x, np.float32).reshape(TOK, D))
    x_pad = np.zeros((NCORES * NT, D), np.float32)
    x_pad[:TOK] = x_np
    x_pad = x_pad.reshape(NCORES, NSUP, 2, 4, 128, D)  # [n, s, c, k, p, d]

    # device-permuted fp32 tokens: row 1024s + 8p + 4c + k
    x_perm = np.ascontiguousarray(x_pad.transpose(0, 1, 4, 2, 3, 5)).reshape(
        NCORES, NT, D
    )
    # transposed bf16 tokens: xt[n, 64c + d, 512s + 128k + p]
    xt16 = np.ascontiguousarray(
        x_pad.astype(ml_dtypes.bfloat16).transpose(0, 2, 5, 1, 3, 4)
    ).reshape(NCORES, 128, NT // 2)

    in_maps = [
        {
            "x": x_perm[n],
            "xt": xt16[n],
            "a": a_np,
            "c": c_np,
            "v": v_np,
        }
        for n in range(NCORES)
    ]

    nc = _build_program()
    res = run_bass_kernel_spmd(nc, in_maps, list(range(NCORES)), trace=TRACE)
    LAST_RESULTS = res

    y = np.stack([res.results[n]["y"] for n in range(NCORES)], axis=0)
    # invert the per-supertile permutation: perm row = 8p + 4c + k
    y = y.reshape(NCORES, NSUP, 128, 2, 4, D).transpose(0, 1, 3, 4, 2, 5)
    y = np.ascontiguousarray(y).reshape(NCORES * NT, D)
    return y[:TOK].reshape(B, L, N, D)


# revision 16
# speedup vs baseline: 1.0861x; 1.0189x over previous
"""Trainium2 Bass kernel for nn_MemoryMultiAttention.

out = x + softmax((x @ Wq + bq) K^T / sqrt(D)) V   per head, tiny shared
memory bank (M=64 slots), H=4 heads of dh=16, D=64.

Strategy:
  * Host folds the Q projection into the score matrix:
        scores[t, h, m] = x[t, :] @ A_h[:, m] + c_h[m]
    with A_h = Wq_h @ K_h^T / 8 (64x64), c_h = bq_h @ K_h^T / 8.
  * Data-parallel over 8 cores: each core handles 1/8 of the B*L*N tokens.
  * The host supplies, per core, both the fp32 tokens (for the residual)
    and a bf16 *transposed* copy laid out [128 = 2 token-halves x 64 d,
    cols] so the scores matmul can contract over d directly; two 64-row
    groups of the PE run concurrently.
  * On device (per supertile of 1024 tokens):
      - TensorE: scoresT[hm, t] = A_pair^T @ xT   (psum [128, 2, 512])
      - ACT: exp(scores + c) with per-partition bias fused; bf16 out
      - TensorE: read_u[t, 0:64] + per-head sumexp[t, 64:68] in one
        accumulated matmul against an augmented block-diagonal V
      - DVE: reciprocal of sums, normalize, add fp32 residual x
  * Token order inside a supertile is permuted so every DMA is 2KB-
    contiguous per partition; the host applies the inverse permutation.
"""

import math
from contextlib import ExitStack

import ml_dtypes
import numpy as np

import concourse.bass as bass
import concourse.mybir as mybir
import concourse.tile as tile
from concourse import bacc
from concourse.bass_utils import run_bass_kernel_spmd

B, L, N, D = 16, 24, 325, 64
M, H = 64, 4
DH = D // H
TOK = B * L * N  # 124800
NCORES = 8
NT = 16384  # padded tokens per core (124800/8 = 15600 -> 16*1024)
NSUP = 16
TS = 1024  # supertile tokens
CH = TS // 128  # 8 chunks of 128 tokens

F32 = mybir.dt.float32
BF16 = mybir.dt.bfloat16

# set by test.py to collect a profile
TRACE = False
LAST_RESULTS = None

_cached_nc = None


def _build_program():
    global _cached_nc
    if _cached_nc is not None:
        return _cached_nc

    nc = bacc.Bacc(
        "TRN2", target_bir_lowering=False, debug=False, num_devices=NCORES
    )
    x_in = nc.declare_dram_parameter("x", [NT, D], F32, isOutput=False)
    xt_in = nc.declare_dram_parameter("xt", [128, NT // 2], BF16, isOutput=False)
    a_in = nc.declare_dram_parameter("a", [128, 2, 128], BF16, isOutput=False)
    c_in = nc.declare_dram_parameter("c", [128, 2], F32, isOutput=False)
    v_in = nc.declare_dram_parameter("v", [128, 2, 68], BF16, isOutput=False)
    y_out = nc.declare_dram_parameter("y", [NT, D], F32, isOutput=True)

    with ExitStack() as ctx:
        tc = ctx.enter_context(tile.TileContext(nc))
        const_pool = ctx.enter_context(tc.tile_pool(name="const", bufs=1))
        xin_pool = ctx.enter_context(tc.tile_pool(name="xin", bufs=4))
        xt_pool = ctx.enter_context(tc.tile_pool(name="xt", bufs=4))
        exp_pool = ctx.enter_context(tc.tile_pool(name="expt", bufs=6))
        o32_pool = ctx.enter_context(tc.tile_pool(name="o32", bufs=3))
        out_pool = ctx.enter_context(tc.tile_pool(name="outp", bufs=3))
        rec_pool = ctx.enter_context(tc.tile_pool(name="recip", bufs=3))
        # psS ([128,2,512] f32) and psR ([128,2,4,128] f32) are both 2 PSUM
        # banks; sharing one 4-slot pool (8 banks) lets the scheduler float
        # the spare slot to whichever side is behind
        ps_pool = ctx.enter_context(tc.tile_pool(name="ps", bufs=4, space="PSUM"))

        # constants, loaded once
        a_t = const_pool.tile([128, 2, 128], BF16)
        nc.sync.dma_start(a_t[:, :, :], a_in[:, :, :])
        c_t = const_pool.tile([128, 2], F32)
        nc.sync.dma_start(c_t[:, :], c_in[:, :])
        v_t = const_pool.tile([128, 2, 68], BF16)
        nc.sync.dma_start(v_t[:, :, :], v_in[:, :, :])

        # dummy exp so the ACT function table loads during the DMA ramp
        # instead of serializing before the first real exp
        warm = const_pool.tile([1, 8], F32)
        nc.vector.memset(warm[:, :], 0.0)
        nc.scalar.activation(
            warm[:, :], warm[:, :], mybir.ActivationFunctionType.Exp
        )

        # software pipeline: scores/exp of supertile s are emitted before the
        # read/normalize phase of supertile s-1 so the PE starts the next
        # scores matmuls as soon as the previous exp drains, keeping ACT fed.
        stage = {}  # s -> (expt pair list, x32 AP)
        outp = {}  # pair idx -> outt tile

        def read_phase(s):
            expt, x32 = stage.pop(s)
            half = s % 2

            # read: chunk cc = 4c + k lives at psR[:, c, k, :];
            # cols 0:64 = read_u, 64:68 = per-head sumexp
            psR = ps_pool.tile([128, 2, 4, 128], F32, tag="ps", name=f"psR{s}")
            for cc in range(CH):
                c, k = cc // 4, cc % 4
                for pp in range(2):
                    nc.tensor.matmul(
                        psR[:, c, k, 0:68],
                        expt[pp][:, c, 128 * k : 128 * (k + 1)],
                        v_t[:, pp, :],
                        start=(pp == 0),
                        stop=(pp == 1),
                    )

            rec = rec_pool.tile([128, 2, 4, 4], F32, tag="rec")
            nc.vector.reciprocal(rec[:, :, :, :], psR[:, :, :, 64:68])

            o32 = o32_pool.tile([128, 2, 4, 4, 16], F32, tag="o32")
            nc.vector.tensor_mul(
                o32[:, :, :, :, :],
                psR[:, :, :, 0:64].rearrange("p b k (h e) -> p b k h e", e=16),
                rec[:, :, :, :].unsqueeze(4).broadcast_to((128, 2, 4, 4, 16)),
            )

            if half == 0:
                outp[s // 2] = out_pool.tile(
                    [128, 2, CH * D], F32, tag="outt", name=f"outt{s}"
                )
            # residual add on the otherwise-idle GpSimd engine (SBUF-only op)
            nc.gpsimd.tensor_add(
                outp[s // 2][:, half],
                o32[:, :, :, :, :].rearrange("p b k h e -> p (b k h e)"),
                x32[:, :],
            )
            if half == 1:
                nc.sync.dma_start(
                    y_out[TS * (s - 1) : TS * (s + 1), :].rearrange(
                        "(u p q) d -> p u (q d)", u=2, p=128
                    ),
                    outp.pop(s // 2)[:, :, :],
                )

        x32_pair = xt_pair = None
        for s in range(NSUP):
            # device token f (col of xt) = 512c + 128k + p; x/y rows are
            # host-permuted so row 1024s + 8p + 4c + k = device token f
            half = s % 2
            if half == 0:
                # one DMA covers two supertiles: bigger descriptors,
                # half the sequencer issue cost; xt first (needed first)
                xt_pair = xt_pool.tile([128, 2, 512], BF16, tag="xt")
                if s == 0:
                    # split the first transfer so scores(0) starts sooner
                    nc.sync.dma_start(xt_pair[:, 0], xt_in[:, 0:512])
                    nc.sync.dma_start(xt_pair[:, 1], xt_in[:, 512:1024])
                else:
                    nc.sync.dma_start(
                        xt_pair[:, :, :],
                        xt_in[:, 512 * s : 512 * (s + 2)].rearrange(
                            "p (u f) -> p u f", u=2
                        ),
                    )
                x32_pair = xin_pool.tile([128, 2, CH * D], F32, tag="x32")
                nc.sync.dma_start(
                    x32_pair[:, :, :],
                    x_in[TS * s : TS * (s + 2), :].rearrange(
                        "(u p q) d -> p u (q d)", u=2, p=128
                    ),
                )
            x32 = x32_pair[:, half]
            xt = xt_pair[:, half]

            # scoresT: psS[pp][hm, (c, f)]
            expt = []
            for pp in range(2):
                ps = ps_pool.tile(
                    [128, 2, 512], F32, tag="ps", name=f"psS{s}_{pp}"
                )
                for c in range(2):
                    nc.tensor.matmul(
                        ps[:, c, :],
                        a_t[64 * c : 64 * (c + 1), pp, :],
                        xt[64 * c : 64 * (c + 1), :],
                        start=True,
                        stop=True,
                    )
                et = exp_pool.tile([128, 2, 512], BF16, tag="expt")
                nc.scalar.activation(
                    et[:, :, :],
                    ps[:, :, :],
                    mybir.ActivationFunctionType.Exp,
                    bias=c_t[:, pp : pp + 1],
                )
                expt.append(et)
            stage[s] = (expt, x32)

            if s > 0:
                read_phase(s - 1)
        read_phase(NSUP - 1)

    nc.compile()
    _cached_nc = nc
    return nc


def _host_constants(memory_bank, Wq, bq, Wk, bk, Wv, bv):
    mb = np.asarray(memory_bank, np.float32)
    Wq = np.asarray(Wq, np.float32)
    bq = np.asarray(bq, np.float32)
    Wk = np.asarray(Wk, np.float32)
    bk = np.asarray(bk, np.float32)
    Wv = np.asarray(Wv, np.float32)
    bv = np.asarray(bv, np.float32)

    K = mb @ Wk + bk  # [M, D]
    V = mb @ Wv + bv  # [M, D]
    scale = 1.0 / math.sqrt(D)

    # a_np[64c + d, pp, j]: A for head (2pp + j//64), slot j%64, replicated c
    a_np = np.zeros((128, 2, 128), np.float32)
    c_np = np.zeros((128, 2), np.float32)
    v_np = np.zeros((128, 2, 68), np.float32)
    for h in range(H):
        Kh = K[:, h * DH : (h + 1) * DH]  # [M, dh]
        Vh = V[:, h * DH : (h + 1) * DH]  # [M, dh]
        Ah = (Wq[:, h * DH : (h + 1) * DH] @ Kh.T) * scale  # [D, M]
        ch = (bq[h * DH : (h + 1) * DH] @ Kh.T) * scale  # [M]
        pp, half = h // 2, h % 2
        for c in range(2):
            a_np[64 * c : 64 * (c + 1), pp, 64 * half : 64 * (half + 1)] = Ah
        q0 = 64 * half
        c_np[q0 : q0 + 64, pp] = ch
        v_np[q0 : q0 + 64, pp, h * DH : (h + 1) * DH] = Vh
        v_np[q0 : q0 + 64, pp, 64 + h] = 1.0

    return (
        a_np.astype(ml_dtypes.bfloat16),
        c_np,
        v_np.astype(ml_dtypes.bfloat16),
    )


def kernel(x, memory_bank, Wq, bq, Wk, bk, Wv, bv):
    global LAST_RESULTS
    a_np, c_np, v_np = _host_constants(memory_bank, Wq, bq, Wk, bk, Wv, bv)

    x_np = np.ascontiguousarray(np.asarray(x, np.float32).reshape(TOK, D))
    x_pad = np.zeros((NCORES * NT, D), np.float32)
    x_pad[:TOK] = x_np
    x_pad = x_pad.reshape(NCORES, NSUP, 2, 4, 128, D)  # [n, s, c, k, p, d]

    # device-permuted fp32 tokens: row 1024s + 8p + 4c + k
    x_perm = np.ascontiguousarray(x_pad.transpose(0, 1, 4, 2, 3, 5)).reshape(
        NCORES, NT, D
    )
    # transposed bf16 tokens: xt[n, 64c + d, 512s + 128k + p]
    xt16 = np.ascontiguousarray(
        x_pad.astype(ml_dtypes.bfloat16).transpose(0, 2, 5, 1, 3, 4)
    ).reshape(NCORES, 128, NT // 2)

    in_maps = [
        {
            "x": x_perm[n],
            "xt": xt16[n],
            "a": a_np,
            "c": c_np,
            "v": v_np,
        }
        for n in range(NCORES)
    ]

    nc = _build_program()
    res = run_bass_kernel_spmd(nc, in_maps, list(range(NCORES)), trace=TRACE)
    LAST_RESULTS = res

    y = np.stack([res.results[n]["y"] for n in range(NCORES)], axis=0)
    # invert the per-supertile permutation: perm row = 8p + 4c + k
    y = y.reshape(NCORES, NSUP, 128, 2, 4, D).transpose(0, 1, 3, 4, 2, 5)
    y = np.ascontiguousarray(y).reshape(NCORES * NT, D)
    return y[:TOK].reshape(B, L, N, D)
